# revision 3
# baseline (speedup 1.0000x reference)
"""GAT layer kernel (v7.1) for Trainium2 (Bass/Tile), data-parallel over batch on 8 cores.

v7.1 vs v6: s computed on DVE (fp32 mult+reduce vs v-broadcast); x
transposed once in bf16; h computed directly in [tok, f] orientation
(xT-block stationary x W moving) killing the fp32 lrl transposes; final
out = Prelu(h*col) fused on ACT (scale=col); token layout (p t) so x/out
DMAs move 2KB+ contiguous per partition; engine queues ordered so masks
start as early as possible (DVE 12 + ACT 4); h matmul chunks 2,3 fill
the PE during the combine phase.

Per-core computation (batch b, N=2048, F=128):
    s = x @ (W @ w_mlp) + b;  p = exp(s), q = exp(0.2 s)
    mt[i,j] = [s_i + s_j > 0] - 1/2   (symmetric, values +-1/2)
    D_i = p_i ((mt p)_i + Ptot/2) + q_i ((mt (-q))_i + Qtot/2)
    col  = p ((mt r) + Rtot/2) + q ((mt (-u)) + Utot/2),  r = p/D, u = q/D
    out  = lrelu(h * col),  h = x @ W
"""

import sys

if "/opt/trn_rl_repo" not in sys.path:
    sys.path.insert(0, "/opt/trn_rl_repo")

from contextlib import ExitStack

import numpy as np

import concourse.bass as bass
import concourse.mybir as mybir
import concourse.tile as tile
from concourse import bacc
from concourse import masks
from concourse.bass_utils import run_bass_kernel_spmd

B, N, F = 8, 2048, 128
NB = N // 128  # 16 token blocks
NC4 = 4  # 512-wide chunks
NEG_SLOPE = 0.2
FP32 = mybir.dt.float32
BF16 = mybir.dt.bfloat16
ALU = mybir.AluOpType
AFT = mybir.ActivationFunctionType

# mask block -> engine: "v" DVE (is_gt - 0.5), "a" ACT (Sign, +-1, halved
# stationary). Emission order matches mv1 consumption readiness.
MASK_ENG = {a: "v" for a in range(NB)}
for a in (4, 9, 11, 14):
    MASK_ENG[a] = "a"
# wave1 (blocks 0-3, split masks) runs c-major and is the start of every
# accumulation region; LAST = the ACT-generated masks (ready earliest of
# the late blocks) consumed c-outer with stop
MV1_WAVE1 = [0, 1, 2, 3]
MV1_ORDER = [8, 5, 6, 7, 12, 13, 10, 15]
MV1_LAST = [4, 9, 14, 11]
# ACT Prelu not implemented in CoreSim; sim_check flips this off.
USE_PRELU = True
# final out blocks engine split (t % 4): "a" ACT Prelu, "v" DVE pair
FIN_ENG = {0: "a", 1: "v", 2: "a", 3: "v"}


def gat_kernel(ctx: ExitStack, tc: "tile.TileContext", out_d, x_d, W_d, wm_d, bm_d):
    nc = tc.nc

    const_p = ctx.enter_context(tc.tile_pool(name="const", bufs=1))
    big_p = ctx.enter_context(tc.tile_pool(name="big", bufs=1))
    mask_p = ctx.enter_context(tc.tile_pool(name="mask", bufs=NB))
    vec_p = ctx.enter_context(tc.tile_pool(name="vec", bufs=1))
    outsb_p = ctx.enter_context(tc.tile_pool(name="outsb", bufs=4))
    # PSUM: bigps 4 banks + tr 1 + tr2 2x1 + sm 1 = 8
    ps_big = ctx.enter_context(tc.tile_pool(name="ps_big", bufs=1, space="PSUM"))
    ps_tr = ctx.enter_context(tc.tile_pool(name="ps_tr", bufs=1, space="PSUM"))
    ps_sm = ctx.enter_context(tc.tile_pool(name="ps_sm", bufs=1, space="PSUM"))

    # iota first on gpsimd: nothing depends upstream and it unblocks the
    # W2 setup on DVE before the s chain claims the queue
    ioti = const_p.tile([128, 1], mybir.dt.int32, tag="ioti")
    nc.gpsimd.iota(ioti[:], [[0, 1]], base=0, channel_multiplier=1)

    # ---------------- input DMAs first (x is the critical path) ----------
    # token layout: tok = p*16 + t  ->  per-partition 2KB-contiguous DMAs
    x_view = x_d.rearrange("(p t) f -> p t f", t=NB)
    x_sb = big_p.tile([128, NB, 128], FP32, tag="x_sb")
    nc.gpsimd.dma_start(x_sb[:, 12:16, :], x_view[:, 12:16, :])
    W_sb = const_p.tile([128, 128], FP32, tag="W_sb")
    nc.sync.dma_start(W_sb[:], W_d[:, :])
    wm_sb = const_p.tile([128, 1], FP32, tag="wm_sb")
    nc.scalar.dma_start(wm_sb[:], wm_d.rearrange("(p o) -> p o", o=1))
    b_sb = const_p.tile([1, 1], FP32, tag="b_sb")
    nc.scalar.dma_start(b_sb[:], bm_d.rearrange("(p o) -> p o", o=1))
    nc.sync.dma_start(x_sb[:, 0:4, :], x_view[:, 0:4, :])
    nc.scalar.dma_start(x_sb[:, 4:8, :], x_view[:, 4:8, :])
    nc.sync.dma_start(x_sb[:, 8:12, :], x_view[:, 8:12, :])
    # identities next on the gpsimd queue: ident_f gates the early W
    # transpose, ident_b the bf16 transposes
    ident_f = const_p.tile([128, 128], FP32, tag="ident_f")
    ident_b = const_p.tile([128, 128], BF16, tag="ident_b")
    masks.make_identity(nc, ident_f[:])
    masks.make_identity(nc, ident_b[:])

    # ---------------- constants ----------------
    ones_f = const_p.tile([128, 1], FP32, tag="ones_f")
    nc.gpsimd.memset(ones_f[:], 1.0)
    ones_row_f = const_p.tile([1, 128], FP32, tag="ones_row_f")
    nc.gpsimd.memset(ones_row_f[:], 1.0)
    halves_row_f = const_p.tile([1, 128], FP32, tag="halves_row_f")
    nc.gpsimd.memset(halves_row_f[:], 0.5)
    ones_row_b = const_p.tile([1, 128], BF16, tag="ones_row_b")
    nc.gpsimd.memset(ones_row_b[:], 1.0)
    # zero-padded 32-col stationaries (cols 4..31 stay 0 so every col-group
    # writes its full 32-partition PSUM range); memsets run on gpsimd after
    # the x casts so they don't delay the transposes
    Pk = vec_p.tile([128, NB, 32], BF16, tag="Pk")
    Pkh = vec_p.tile([128, NB, 32], BF16, tag="Pkh")
    Rk = vec_p.tile([128, NB, 32], BF16, tag="Rk")
    Rkh = vec_p.tile([128, NB, 32], BF16, tag="Rkh")

    # W2 [128, 2]: col0 selects rows {32j+0,32j+1}, col1 rows {32j+2,32j+3}
    W2 = const_p.tile([128, 2], FP32, tag="W2")
    m32i = const_p.tile([128, 1], mybir.dt.int32, tag="m32i")
    nc.vector.tensor_scalar(m32i[:], ioti[:], 31, None, ALU.bitwise_and)
    m32 = const_p.tile([128, 1], FP32, tag="m32")
    nc.vector.tensor_copy(m32[:], m32i[:])
    nc.vector.tensor_scalar(W2[:, 0:1], m32[:], 2.0, None, ALU.is_lt)
    lt4 = const_p.tile([128, 1], FP32, tag="lt4")
    nc.vector.tensor_scalar(lt4[:], m32[:], 4.0, None, ALU.is_lt)
    nc.vector.tensor_tensor(W2[:, 1:2], lt4[:], W2[:, 0:1], ALU.subtract)

    # Preload the ACT table set early (exp_and_others also holds sign, copy,
    # parametric_relu)
    warm = const_p.tile([128, 2], FP32, tag="warm")
    nc.scalar.activation(warm[:, 0:1], ones_f[:], AFT.Exp)
    nc.scalar.activation(warm[:, 1:2], ones_f[:], AFT.Sign)

    # ---------------- v row = (W @ w_mlp)^T, broadcast to [128, 4, 128] --
    # vrow[f] = sum_g wm[g] W[f, g] = wm^T @ W^T
    WT_ps = ps_sm.tile([128, 128], FP32, tag="sm")
    nc.tensor.transpose(WT_ps[:], W_sb[:], ident_f[:])
    WT_sb = vec_p.tile([128, 128], FP32, tag="WT_sb")
    nc.vector.tensor_copy(WT_sb[:], WT_ps[:])
    vrow_ps = ps_sm.tile([1, 128], FP32, tag="sm")
    nc.tensor.matmul(vrow_ps[:], lhsT=wm_sb[:], rhs=WT_sb[:], start=True, stop=True)
    vrow_sb = vec_p.tile([1, 128], FP32, tag="vrow_sb")
    nc.vector.tensor_copy(vrow_sb[:], vrow_ps[:])
    # v broadcast down partitions (single width; the s products read it
    # from PSUM through a stride-0 x4 view)
    vbc_ps = ps_tr.tile([128, 128], FP32, tag="tr")
    nc.tensor.matmul(
        vbc_ps[:], lhsT=ones_row_f[:], rhs=vrow_sb[:], start=True, stop=True
    )
    vbc4_view = vbc_ps.rearrange("p (o b) -> p o b", o=1).broadcast_to(
        (128, 4, 128)
    )

    # b broadcast to [128,1] via K=1 PE matmul
    b_ps = ps_sm.tile([128, 1], FP32, tag="sm")
    nc.tensor.matmul(b_ps[:], lhsT=ones_row_f[:], rhs=b_sb[:], start=True, stop=True)
    b_bc = const_p.tile([128, 1], FP32, tag="b_bc")
    nc.vector.tensor_copy(b_bc[:], b_ps[:])

    # W in bf16 (single pass; rel-err budget 2e-2 allows it)
    W_hi = const_p.tile([128, 128], BF16, tag="W_hi")
    nc.scalar.copy(W_hi[:], W_sb[:])

    # ---------------- per-half: cast x, s on DVE, s-chain, xT -------------
    x_bf = big_p.tile([128, NB, 128], BF16, tag="x_bf")
    xT = big_p.tile([128, NB, 128], BF16, tag="xT")
    s_mat = vec_p.tile([128, NB], FP32, tag="s_mat")
    s_prod = vec_p.tile([128, 4, 128], FP32, tag="s_prod")
    s_hi = vec_p.tile([128, NB], BF16, tag="s_hi")
    neg_s = vec_p.tile([128, NB], FP32, tag="neg_s")
    s_flat = vec_p.tile([1, N], BF16, tag="s_flat")
    S_row = big_p.tile([128, N], BF16, tag="S_row")

    def eng_copy(e, dst, src):
        (e.copy if e is nc.scalar else e.tensor_copy)(dst, src)

    cast_eng = [nc.vector, nc.scalar, nc.vector, nc.scalar]

    for hh in range(2):
        for c in (2 * hh, 2 * hh + 1):
            # s products first: the casts must not head-of-line-block them
            cs = slice(4 * c, 4 * c + 4)
            nc.vector.tensor_tensor(s_prod[:], x_sb[:, cs, :], vbc4_view, ALU.mult)
            nc.vector.reduce_sum(s_mat[:, cs], s_prod[:], axis=mybir.AxisListType.X)
            for h2 in range(2):
                sl = slice(4 * c + 2 * h2, 4 * c + 2 * h2 + 2)
                eng_copy(cast_eng[(2 * c + h2) % 4], x_bf[:, sl, :], x_sb[:, sl, :])
        # s chain: s_flat row + S_row broadcast for this half
        bs = slice(hh * 8, (hh + 1) * 8)
        nc.vector.tensor_scalar(
            s_mat[:, bs], s_mat[:, bs], b_bc[:, 0:1], None, ALU.add
        )
        nc.vector.tensor_copy(s_hi[:, bs], s_mat[:, bs])
        sT_ps = ps_sm.tile([8, 128], BF16, tag="sm")
        nc.tensor.transpose(sT_ps[:], s_hi[:, bs], ident_b[:])
        sT_sb = vec_p.tile([8, 128], BF16, tag=f"sT_sb{hh}")
        nc.vector.tensor_copy(sT_sb[:], sT_ps[:])
        nc.sync.dma_start(s_flat[0:1, hh * 1024 : (hh + 1) * 1024], sT_sb[:, :])
        for k in range(2):
            c = 2 * hh + k
            sl = slice(c * 512, (c + 1) * 512)
            S_ps = ps_tr.tile([128, 512], FP32, tag="tr2", bufs=2)
            nc.tensor.matmul(
                S_ps[:], lhsT=ones_row_b[:], rhs=s_flat[0:1, sl],
                start=True, stop=True,
            )
            nc.scalar.copy(S_row[:, sl], S_ps[:])
        nc.vector.tensor_scalar(neg_s[:, bs], s_mat[:, bs], -1.0, None, ALU.mult)
    for stat in (Pk, Pkh, Rk, Rkh):
        nc.gpsimd.memset(stat[:], 0.0)

    # p = exp(s), q = exp(0.2 s) on ACT
    p_v = vec_p.tile([128, NB], FP32, tag="p_v")
    nc.scalar.activation(p_v[:], s_mat[:], AFT.Exp)
    q_v = vec_p.tile([128, NB], FP32, tag="q_v")
    nc.scalar.activation(q_v[:], s_mat[:], AFT.Exp, scale=NEG_SLOPE)

    # ---------------- masks (emitted in consumption order) ----------------
    mask_tiles = [mask_p.tile([128, N], BF16, tag="mask", name=f"mask{a}")
                  for a in range(NB)]

    def mask_dve(a, lo, hi):
        nc.vector.tensor_scalar(
            mask_tiles[a][:, lo:hi], S_row[:, lo:hi], neg_s[:, a : a + 1], 0.5,
            ALU.is_gt, ALU.subtract,
        )

    # DVE: split blocks 0-3 (left halves start on S_row[:, :1024])
    for a in (0, 1, 2, 3):
        mask_dve(a, 0, 1024)

    # xT (bf16) via PE transposes, after the s products (the "tr" psum ring
    # recycles vbc4's bank, so these must come after all its readers);
    # copies on DVE between the mask waves
    for hh in range(2):
        tp = ps_tr.tile([128, 1024], BF16, tag="tr")
        for k in range(8):
            t = 8 * hh + k
            nc.tensor.matmul(
                tp[:, k * 128 : (k + 1) * 128], lhsT=x_bf[:, t, :], rhs=ident_b[:],
                is_transpose=True, start=(k % 2 == 0), stop=(k % 2 == 1),
            )
        nc.vector.tensor_copy(xT[:, 8 * hh : 8 * hh + 8, :], tp[:])

    for a in (0, 1, 2, 3):
        mask_dve(a, 1024, 2048)
    # ACT masks (emitted now so they start right after the exps)
    for a in (4, 9, 11, 14):
        nc.scalar.activation(
            mask_tiles[a][:], S_row[:], AFT.Sign, bias=s_mat[:, a : a + 1]
        )

    # Pk packing (DVE, after exps)
    nc.vector.tensor_copy(Pk[:, :, 0], p_v[:])
    p_hi32 = vec_p.tile([128, NB], FP32, tag="p_hi32")
    nc.vector.tensor_copy(p_hi32[:], Pk[:, :, 0])
    nc.vector.tensor_tensor(Pk[:, :, 1], p_v[:], p_hi32[:], ALU.subtract)
    nc.vector.tensor_scalar(Pk[:, :, 2], q_v[:], -1.0, None, ALU.mult)
    qn_hi32 = vec_p.tile([128, NB], FP32, tag="qn_hi32")
    nc.vector.tensor_copy(qn_hi32[:], Pk[:, :, 2])
    nc.vector.scalar_tensor_tensor(
        Pk[:, :, 3], qn_hi32[:], -1.0, q_v[:], ALU.mult, ALU.subtract
    )
    nc.vector.tensor_scalar(Pkh[:, :, 0:4], Pk[:, :, 0:4], 0.5, None, ALU.mult)

    # Ptot/Qtot -> half-total broadcast htot [128, 2] = [Ptot/2, Qtot/2]
    # (before the full-width masks so the PE-side matmuls don't stall the
    # PE queue behind the DVE mask stream)
    pq_s = vec_p.tile([128, 2], FP32, tag="pq_s")
    nc.vector.reduce_sum(pq_s[:, 0:1], p_v[:], axis=mybir.AxisListType.X)
    nc.vector.reduce_sum(pq_s[:, 1:2], q_v[:], axis=mybir.AxisListType.X)
    tot_ps = ps_sm.tile([1, 2], FP32, tag="sm")
    nc.tensor.matmul(tot_ps[:], lhsT=ones_f[:], rhs=pq_s[:], start=True, stop=True)
    tot_sb = vec_p.tile([1, 2], FP32, tag="tot_sb")
    nc.vector.tensor_copy(tot_sb[:], tot_ps[:])
    htot_ps = ps_sm.tile([128, 2], FP32, tag="sm")
    nc.tensor.matmul(
        htot_ps[:], lhsT=halves_row_f[:], rhs=tot_sb[:], start=True, stop=True
    )
    htot = vec_p.tile([128, 2], FP32, tag="htot")
    nc.vector.tensor_copy(htot[:], htot_ps[:])

    # remaining DVE masks, full width, in mv1 consumption order
    for a in (8, 5, 6, 7, 12, 13, 10, 15):
        mask_dve(a, 0, 2048)

    # ---------------- h = x @ W in [tok, f] orientation -------------------
    # lhsT = xT block (stationary), rhs = W_hi (moving); copies deferred so
    # they don't block the DVE mask stream
    h_sb = big_p.tile([128, NB, 128], BF16, tag="h_sb")
    h_ps_tiles = {}

    def h_mm(c):
        hp = ps_tr.tile([128, 4, 128], FP32, tag="tr2", bufs=2)
        h_ps_tiles[c] = hp
        for tt in range(4):
            t = 4 * c + tt
            nc.tensor.matmul(
                hp[:, tt, :], lhsT=xT[:, t, :], rhs=W_hi[:], start=True, stop=True
            )

    def h_copy(c):
        nc.vector.tensor_copy(h_sb[:, 4 * c : 4 * c + 4, :], h_ps_tiles[c][:])

    def stat_for(base, base_h, a):
        return (base_h if MASK_ENG[a] == "a" else base)[:, a, :]

    # ---------------- matvec 1 (col-tiled; last round c-outer) ------------
    d_ps = ps_big.tile([128, N], FP32, tag="bigps")

    def mv1_mm(a, c, stop):
        j = a % 4
        nc.tensor.matmul(
            d_ps[32 * j : 32 * j + 32, c * 512 : (c + 1) * 512],
            lhsT=stat_for(Pk, Pkh, a),
            rhs=mask_tiles[a][:, c * 512 : (c + 1) * 512],
            start=(a in MV1_WAVE1),
            stop=stop,
            tile_position=(0, 32 * j),
            skip_group_check=True,
        )

    # wave1 c-major over the split blocks: left chunks stream while the
    # right mask halves are still being generated; h matmuls fill the
    # PE while the full-width masks are produced
    for c in (0, 1):
        for a in MV1_WAVE1:
            mv1_mm(a, c, False)
    h_mm(0)
    for c in (2, 3):
        for a in MV1_WAVE1:
            mv1_mm(a, c, False)
    h_mm(1)
    for a in MV1_ORDER:
        for c in range(NC4):
            mv1_mm(a, c, False)

    # deferred h copies (DVE, after the mask stream)
    h_copy(0)
    h_copy(1)

    # per-chunk tail: copy psum chunk to SBUF, then one K=128 reduction
    # matmul per token block: csb_block^T @ W2 -> [128 tok, 2] partials
    Dt2_ps = ps_sm.tile([128, NB, 2], FP32, tag="sm")

    def mv_tail_chunk(src_ps, dst2_ps, c, copy1):
        csb = vec_p.tile([128, 512], FP32, tag=f"csb{c % 2}")
        copy1(csb[:], src_ps[:, c * 512 : (c + 1) * 512])
        for tt in range(4):
            t = c * 4 + tt
            nc.tensor.matmul(
                dst2_ps[:, t, :],
                lhsT=csb[:, tt * 128 : (tt + 1) * 128],
                rhs=W2[:],
                start=True,
                stop=True,
            )

    # tails one chunk behind the LAST rounds so the W2 matmuls never make
    # the PE wait on the csb copies
    def mv1_tail(c):
        mv_tail_chunk(
            d_ps, Dt2_ps, c,
            nc.vector.tensor_copy if c % 2 == 0 else nc.scalar.copy,
        )

    for c in range(NC4):
        for j in range(4):
            mv1_mm(MV1_LAST[j], c, True)
        if c >= 1:
            mv1_tail(c - 1)
    mv1_tail(3)

    # late h chunks fill the PE while the combine chain runs on DVE
    h_mm(2)
    h_mm(3)

    Dt2 = vec_p.tile([128, NB, 2], FP32, tag="Dt2")
    nc.vector.tensor_copy(Dt2[:], Dt2_ps[:])

    # ---------------- combine: A/B [128, NB], D, Rk ------------------------
    A_v = vec_p.tile([128, NB], FP32, tag="A_v")
    nc.vector.tensor_scalar(A_v[:], Dt2[:, :, 0], htot[:, 0:1], None, ALU.add)
    B_v = vec_p.tile([128, NB], FP32, tag="B_v")
    nc.vector.tensor_scalar(B_v[:], Dt2[:, :, 1], htot[:, 1:2], None, ALU.add)
    t1 = vec_p.tile([128, NB], FP32, tag="t1")
    nc.vector.tensor_tensor(t1[:], p_v[:], A_v[:], ALU.mult)
    t2 = vec_p.tile([128, NB], FP32, tag="t2")
    nc.vector.tensor_tensor(t2[:], q_v[:], B_v[:], ALU.mult)
    D_v = vec_p.tile([128, NB], FP32, tag="D_v")
    nc.vector.tensor_tensor(D_v[:], t1[:], t2[:], ALU.add)
    invD = vec_p.tile([128, NB], FP32, tag="invD")
    nc.vector.reciprocal(invD[:], D_v[:])
    r_v = vec_p.tile([128, NB], FP32, tag="r_v")
    nc.vector.tensor_tensor(r_v[:], p_v[:], invD[:], ALU.mult)
    u_v = vec_p.tile([128, NB], FP32, tag="u_v")
    nc.vector.tensor_tensor(u_v[:], q_v[:], invD[:], ALU.mult)
    nc.vector.tensor_copy(Rk[:, :, 0], r_v[:])
    r_hi32 = vec_p.tile([128, NB], FP32, tag="r_hi32")
    nc.vector.tensor_copy(r_hi32[:], Rk[:, :, 0])
    nc.vector.tensor_tensor(Rk[:, :, 1], r_v[:], r_hi32[:], ALU.subtract)
    nc.vector.tensor_scalar(Rk[:, :, 2], u_v[:], -1.0, None, ALU.mult)
    un_hi32 = vec_p.tile([128, NB], FP32, tag="un_hi32")
    nc.vector.tensor_copy(un_hi32[:], Rk[:, :, 2])
    nc.vector.scalar_tensor_tensor(
        Rk[:, :, 3], un_hi32[:], -1.0, u_v[:], ALU.mult, ALU.subtract
    )
    nc.vector.tensor_scalar(Rkh[:, :, 0:4], Rk[:, :, 0:4], 0.5, None, ALU.mult)

    # Rtot/Utot -> half-total broadcast htot2 [128, 2]
    ru_s = vec_p.tile([128, 2], FP32, tag="ru_s")
    nc.vector.reduce_sum(ru_s[:, 0:1], r_v[:], axis=mybir.AxisListType.X)
    nc.vector.reduce_sum(ru_s[:, 1:2], u_v[:], axis=mybir.AxisListType.X)
    tot2_ps = ps_sm.tile([1, 2], FP32, tag="sm")
    nc.tensor.matmul(tot2_ps[:], lhsT=ones_f[:], rhs=ru_s[:], start=True, stop=True)
    tot2_sb = vec_p.tile([1, 2], FP32, tag="tot2_sb")
    nc.vector.tensor_copy(tot2_sb[:], tot2_ps[:])
    htot2_ps = ps_sm.tile([128, 2], FP32, tag="sm")
    nc.tensor.matmul(
        htot2_ps[:], lhsT=halves_row_f[:], rhs=tot2_sb[:], start=True, stop=True
    )
    htot2 = vec_p.tile([128, 2], FP32, tag="htot2")
    nc.vector.tensor_copy(htot2[:], htot2_ps[:])

    # deferred h copies for chunks 2,3 (combine window, after the htot2
    # chain so they don't delay the mv2 start)
    h_copy(2)
    h_copy(3)

    # ---------------- matvec 2 (c-outer, col-tiled, pipelined tails) ------
    out_view = out_d.rearrange("(p t) f -> p t f", t=NB)
    g_ps = ps_big.tile([128, N], FP32, tag="bigps")
    Gt2_ps = ps_sm.tile([128, NB, 2], FP32, tag="sm")
    gsb = vec_p.tile([128, NB, 2], FP32, tag="gsb")
    gA = vec_p.tile([128, NB], FP32, tag="gA")
    gB = vec_p.tile([128, NB], FP32, tag="gB")
    gt1 = vec_p.tile([128, NB], FP32, tag="gt1")
    col = vec_p.tile([128, NB], FP32, tag="col")
    fin_tmp = vec_p.tile([128, 4, 128], FP32, tag="fin_tmp")

    def mv2_chunk(c):
        for rr in range(NC4):
            for j in range(4):
                a = 4 * rr + j
                nc.tensor.matmul(
                    g_ps[32 * j : 32 * j + 32, c * 512 : (c + 1) * 512],
                    lhsT=stat_for(Rk, Rkh, a),
                    rhs=mask_tiles[a][:, c * 512 : (c + 1) * 512],
                    start=(rr == 0),
                    stop=(rr == 3),
                    tile_position=(0, 32 * j),
                    skip_group_check=True,
                )

    def fin_block(o_sb, tt, t):
        """out_block = lrelu(h * col): ACT fused Prelu, or DVE pair."""
        eng = FIN_ENG[tt] if USE_PRELU else "v"
        if eng == "a":
            nc.scalar.activation(
                o_sb[:, tt, :], h_sb[:, t, :], AFT.Prelu,
                scale=col[:, t : t + 1], alpha=NEG_SLOPE,
            )
        else:
            tmp = fin_tmp[:, tt, :]
            nc.vector.tensor_scalar(
                tmp[:], h_sb[:, t, :], col[:, t : t + 1], None, ALU.mult
            )
            nc.vector.scalar_tensor_tensor(
                o_sb[:, tt, :], tmp[:], NEG_SLOPE, tmp[:], ALU.mult, ALU.max
            )

    def mv2_tail(c):
        mv_tail_chunk(
            g_ps, Gt2_ps, c,
            nc.vector.tensor_copy if c % 2 == 0 else nc.scalar.copy,
        )
        ts = slice(c * 4, (c + 1) * 4)
        nc.vector.tensor_copy(gsb[:, ts, :], Gt2_ps[:, ts, :])
        nc.vector.tensor_scalar(gA[:, ts], gsb[:, ts, 0], htot2[:, 0:1], None, ALU.add)
        nc.vector.tensor_scalar(gB[:, ts], gsb[:, ts, 1], htot2[:, 1:2], None, ALU.add)
        nc.vector.tensor_tensor(gt1[:, ts], p_v[:, ts], gA[:, ts], ALU.mult)
        nc.vector.tensor_tensor(col[:, ts], q_v[:, ts], gB[:, ts], ALU.mult)
        nc.vector.tensor_tensor(col[:, ts], col[:, ts], gt1[:, ts], ALU.add)
        o_sb = outsb_p.tile([128, 4, 128], FP32, tag="o_sb")
        for tt in range(4):
            fin_block(o_sb, tt, c * 4 + tt)
        (nc.sync if c % 2 == 0 else nc.gpsimd).dma_start(
            out_view[:, c * 4 : (c + 1) * 4, :], o_sb[:]
        )

    mv2_chunk(0)
    mv2_chunk(1)
    mv2_tail(0)
    mv2_chunk(2)
    mv2_tail(1)
    mv2_chunk(3)
    mv2_tail(2)
    mv2_tail(3)


def build_nc(num_devices: int = 8) -> "bass.Bass":
    nc = bacc.Bacc(
        "TRN2", target_bir_lowering=False, debug=False, num_devices=num_devices
    )
    x_d = nc.dram_tensor("x", [N, F], FP32, kind="ExternalInput")
    W_d = nc.dram_tensor("W", [F, F], FP32, kind="ExternalInput")
    wm_d = nc.dram_tensor("w_mlp", [F], FP32, kind="ExternalInput")
    bm_d = nc.dram_tensor("b_mlp", [1], FP32, kind="ExternalInput")
    out_d = nc.dram_tensor("out", [N, F], FP32, kind="ExternalOutput")
    with tile.TileContext(nc) as tc:
        with ExitStack() as ctx:
            gat_kernel(ctx, tc, out_d.ap(), x_d.ap(), W_d.ap(), wm_d.ap(), bm_d.ap())
    nc.compile()
    return nc


_NC_CACHE: dict = {}


def run(x, W, w_mlp, b_mlp, trace=False, **spmd_kwargs):
    x = np.asarray(x, dtype=np.float32)
    W = np.asarray(W, dtype=np.float32)
    w_mlp = np.asarray(w_mlp, dtype=np.float32)
    b_mlp = np.asarray(b_mlp, dtype=np.float32)

    if "nc" not in _NC_CACHE:
        _NC_CACHE["nc"] = build_nc(num_devices=B)
    nc = _NC_CACHE["nc"]

    in_maps = [
        {"x": np.ascontiguousarray(x[b, 0]), "W": W, "w_mlp": w_mlp, "b_mlp": b_mlp}
        for b in range(B)
    ]
    res = run_bass_kernel_spmd(
        nc, in_maps, core_ids=list(range(B)), trace=trace, **spmd_kwargs
    )
    out = np.stack([res.results[b]["out"] for b in range(B)])[:, None]
    return out.astype(np.float32), res


def kernel(x, W, w_mlp, b_mlp):
    out, _ = run(x, W, w_mlp, b_mlp)
    return out


# revision 4
# speedup vs baseline: 1.0278x; 1.0278x over previous
"""GAT layer kernel (v7.1) for Trainium2 (Bass/Tile), data-parallel over batch on 8 cores.

v7.1 vs v6: s computed on DVE (fp32 mult+reduce vs v-broadcast); x
transposed once in bf16; h computed directly in [tok, f] orientation
(xT-block stationary x W moving) killing the fp32 lrl transposes; final
out = Prelu(h*col) fused on ACT (scale=col); token layout (p t) so x/out
DMAs move 2KB+ contiguous per partition; engine queues ordered so masks
start as early as possible (DVE 12 + ACT 4); h matmul chunks 2,3 fill
the PE during the combine phase.

Per-core computation (batch b, N=2048, F=128):
    s = x @ (W @ w_mlp) + b;  p = exp(s), q = exp(0.2 s)
    mt[i,j] = [s_i + s_j > 0] - 1/2   (symmetric, values +-1/2)
    D_i = p_i ((mt p)_i + Ptot/2) + q_i ((mt (-q))_i + Qtot/2)
    col  = p ((mt r) + Rtot/2) + q ((mt (-u)) + Utot/2),  r = p/D, u = q/D
    out  = lrelu(h * col),  h = x @ W
"""

import sys

if "/opt/trn_rl_repo" not in sys.path:
    sys.path.insert(0, "/opt/trn_rl_repo")

from contextlib import ExitStack

import numpy as np

import concourse.bass as bass
import concourse.mybir as mybir
import concourse.tile as tile
from concourse import bacc
from concourse import masks
from concourse.bass_utils import run_bass_kernel_spmd

B, N, F = 8, 2048, 128
NB = N // 128  # 16 token blocks
NC4 = 4  # 512-wide chunks
NEG_SLOPE = 0.2
FP32 = mybir.dt.float32
BF16 = mybir.dt.bfloat16
ALU = mybir.AluOpType
AFT = mybir.ActivationFunctionType

# mask block -> engine: "v" DVE (is_gt - 0.5), "a" ACT (Sign, +-1, halved
# stationary). Emission order matches mv1 consumption readiness.
MASK_ENG = {a: "v" for a in range(NB)}
for a in (4, 9, 11, 14):
    MASK_ENG[a] = "a"
# wave1 (blocks 0-3, split masks) runs c-major and is the start of every
# accumulation region; LAST = the ACT-generated masks (ready earliest of
# the late blocks) consumed c-outer with stop
MV1_WAVE1 = [0, 1, 2, 3]
MV1_ORDER = [8, 5, 6, 7, 12, 13, 10, 15]
MV1_LAST = [4, 9, 14, 11]
# ACT Prelu not implemented in CoreSim; sim_check flips this off.
USE_PRELU = True
# final out blocks engine split (t % 4): "a" ACT Prelu, "v" DVE pair
FIN_ENG = {0: "a", 1: "v", 2: "a", 3: "v"}


def gat_kernel(ctx: ExitStack, tc: "tile.TileContext", out_d, x_d, W_d, wm_d, bm_d):
    nc = tc.nc

    const_p = ctx.enter_context(tc.tile_pool(name="const", bufs=1))
    big_p = ctx.enter_context(tc.tile_pool(name="big", bufs=1))
    mask_p = ctx.enter_context(tc.tile_pool(name="mask", bufs=NB))
    vec_p = ctx.enter_context(tc.tile_pool(name="vec", bufs=1))
    outsb_p = ctx.enter_context(tc.tile_pool(name="outsb", bufs=4))
    # PSUM: bigps 4 banks + tr 1 + tr2 2x1 + sm 1 = 8
    ps_big = ctx.enter_context(tc.tile_pool(name="ps_big", bufs=1, space="PSUM"))
    ps_tr = ctx.enter_context(tc.tile_pool(name="ps_tr", bufs=1, space="PSUM"))
    ps_sm = ctx.enter_context(tc.tile_pool(name="ps_sm", bufs=1, space="PSUM"))

    # iota first on gpsimd: nothing depends upstream and it unblocks the
    # W2 setup on DVE before the s chain claims the queue
    ioti = const_p.tile([128, 1], mybir.dt.int32, tag="ioti")
    nc.gpsimd.iota(ioti[:], [[0, 1]], base=0, channel_multiplier=1)

    # ---------------- input DMAs first (x is the critical path) ----------
    # token layout: tok = p*16 + t  ->  per-partition 2KB-contiguous DMAs
    x_view = x_d.rearrange("(p t) f -> p t f", t=NB)
    x_sb = big_p.tile([128, NB, 128], FP32, tag="x_sb")
    W_sb = const_p.tile([128, 128], FP32, tag="W_sb")
    nc.sync.dma_start(W_sb[:], W_d[:, :])
    wm_sb = const_p.tile([128, 1], FP32, tag="wm_sb")
    nc.scalar.dma_start(wm_sb[:], wm_d.rearrange("(p o) -> p o", o=1))
    b_sb = const_p.tile([1, 1], FP32, tag="b_sb")
    nc.scalar.dma_start(b_sb[:], bm_d.rearrange("(p o) -> p o", o=1))
    nc.sync.dma_start(x_sb[:, 0:4, :], x_view[:, 0:4, :])
    nc.scalar.dma_start(x_sb[:, 4:8, :], x_view[:, 4:8, :])
    nc.sync.dma_start(x_sb[:, 8:12, :], x_view[:, 8:12, :])
    # x3 on scalar, outputs on sync: gpsimd issues no DMAs at all, so its
    # (slow, ~2.5us) software-DGE drain never sits at the end of the kernel
    nc.scalar.dma_start(x_sb[:, 12:16, :], x_view[:, 12:16, :])
    # identities next on the gpsimd queue: ident_f gates the early W
    # transpose, ident_b the bf16 transposes
    ident_f = const_p.tile([128, 128], FP32, tag="ident_f")
    ident_b = const_p.tile([128, 128], BF16, tag="ident_b")
    masks.make_identity(nc, ident_f[:])
    masks.make_identity(nc, ident_b[:])

    # ---------------- constants ----------------
    ones_f = const_p.tile([128, 1], FP32, tag="ones_f")
    nc.gpsimd.memset(ones_f[:], 1.0)
    ones_row_f = const_p.tile([1, 128], FP32, tag="ones_row_f")
    nc.gpsimd.memset(ones_row_f[:], 1.0)
    halves_row_f = const_p.tile([1, 128], FP32, tag="halves_row_f")
    nc.gpsimd.memset(halves_row_f[:], 0.5)
    ones_row_b = const_p.tile([1, 128], BF16, tag="ones_row_b")
    nc.gpsimd.memset(ones_row_b[:], 1.0)
    # zero-padded 32-col stationaries (cols 4..31 stay 0 so every col-group
    # writes its full 32-partition PSUM range); memsets run on gpsimd after
    # the x casts so they don't delay the transposes
    Pk = vec_p.tile([128, NB, 32], BF16, tag="Pk")
    Pkh = vec_p.tile([128, NB, 32], BF16, tag="Pkh")
    Rk = vec_p.tile([128, NB, 32], BF16, tag="Rk")
    Rkh = vec_p.tile([128, NB, 32], BF16, tag="Rkh")

    # W2 [128, 2]: col0 selects rows {32j+0,32j+1}, col1 rows {32j+2,32j+3}
    W2 = const_p.tile([128, 2], FP32, tag="W2")
    m32i = const_p.tile([128, 1], mybir.dt.int32, tag="m32i")
    nc.vector.tensor_scalar(m32i[:], ioti[:], 31, None, ALU.bitwise_and)
    m32 = const_p.tile([128, 1], FP32, tag="m32")
    nc.vector.tensor_copy(m32[:], m32i[:])
    nc.vector.tensor_scalar(W2[:, 0:1], m32[:], 2.0, None, ALU.is_lt)
    lt4 = const_p.tile([128, 1], FP32, tag="lt4")
    nc.vector.tensor_scalar(lt4[:], m32[:], 4.0, None, ALU.is_lt)
    nc.vector.tensor_tensor(W2[:, 1:2], lt4[:], W2[:, 0:1], ALU.subtract)

    # Preload the ACT table set early (exp_and_others also holds sign, copy,
    # parametric_relu)
    warm = const_p.tile([128, 2], FP32, tag="warm")
    nc.scalar.activation(warm[:, 0:1], ones_f[:], AFT.Exp)
    nc.scalar.activation(warm[:, 1:2], ones_f[:], AFT.Sign)

    # ---------------- v row = (W @ w_mlp)^T, broadcast to [128, 4, 128] --
    # vrow[f] = sum_g wm[g] W[f, g] = wm^T @ W^T
    WT_ps = ps_sm.tile([128, 128], FP32, tag="sm")
    nc.tensor.transpose(WT_ps[:], W_sb[:], ident_f[:])
    WT_sb = vec_p.tile([128, 128], FP32, tag="WT_sb")
    nc.vector.tensor_copy(WT_sb[:], WT_ps[:])
    vrow_ps = ps_sm.tile([1, 128], FP32, tag="sm")
    nc.tensor.matmul(vrow_ps[:], lhsT=wm_sb[:], rhs=WT_sb[:], start=True, stop=True)
    vrow_sb = vec_p.tile([1, 128], FP32, tag="vrow_sb")
    nc.vector.tensor_copy(vrow_sb[:], vrow_ps[:])
    # v broadcast down partitions (single width; the s products read it
    # from PSUM through a stride-0 x4 view)
    vbc_ps = ps_tr.tile([128, 128], FP32, tag="tr")
    nc.tensor.matmul(
        vbc_ps[:], lhsT=ones_row_f[:], rhs=vrow_sb[:], start=True, stop=True
    )
    vbc4_view = vbc_ps.rearrange("p (o b) -> p o b", o=1).broadcast_to(
        (128, 4, 128)
    )

    # b broadcast to [128,1] via K=1 PE matmul
    b_ps = ps_sm.tile([128, 1], FP32, tag="sm")
    nc.tensor.matmul(b_ps[:], lhsT=ones_row_f[:], rhs=b_sb[:], start=True, stop=True)
    b_bc = const_p.tile([128, 1], FP32, tag="b_bc")
    nc.vector.tensor_copy(b_bc[:], b_ps[:])

    # W in bf16 (single pass; rel-err budget 2e-2 allows it)
    W_hi = const_p.tile([128, 128], BF16, tag="W_hi")
    nc.scalar.copy(W_hi[:], W_sb[:])

    # ---------------- per-half: cast x, s on DVE, s-chain, xT -------------
    x_bf = big_p.tile([128, NB, 128], BF16, tag="x_bf")
    xT = big_p.tile([128, NB, 128], BF16, tag="xT")
    s_mat = vec_p.tile([128, NB], FP32, tag="s_mat")
    s_prod = vec_p.tile([128, 4, 128], FP32, tag="s_prod")
    s_hi = vec_p.tile([128, NB], BF16, tag="s_hi")
    neg_s = vec_p.tile([128, NB], FP32, tag="neg_s")
    s_flat = vec_p.tile([1, N], BF16, tag="s_flat")
    S_row = big_p.tile([128, N], BF16, tag="S_row")

    def eng_copy(e, dst, src):
        (e.copy if e is nc.scalar else e.tensor_copy)(dst, src)

    cast_eng = [nc.vector, nc.scalar, nc.vector, nc.scalar]

    for hh in range(2):
        for c in (2 * hh, 2 * hh + 1):
            # s products first: the casts must not head-of-line-block them
            cs = slice(4 * c, 4 * c + 4)
            nc.vector.tensor_tensor(s_prod[:], x_sb[:, cs, :], vbc4_view, ALU.mult)
            nc.vector.reduce_sum(s_mat[:, cs], s_prod[:], axis=mybir.AxisListType.X)
            for h2 in range(2):
                sl = slice(4 * c + 2 * h2, 4 * c + 2 * h2 + 2)
                eng_copy(cast_eng[(2 * c + h2) % 4], x_bf[:, sl, :], x_sb[:, sl, :])
        # s chain: s_flat row + S_row broadcast for this half
        bs = slice(hh * 8, (hh + 1) * 8)
        nc.vector.tensor_scalar(
            s_mat[:, bs], s_mat[:, bs], b_bc[:, 0:1], None, ALU.add
        )
        nc.vector.tensor_copy(s_hi[:, bs], s_mat[:, bs])
        sT_ps = ps_sm.tile([8, 128], BF16, tag="sm")
        nc.tensor.transpose(sT_ps[:], s_hi[:, bs], ident_b[:])
        sT_sb = vec_p.tile([8, 128], BF16, tag=f"sT_sb{hh}")
        nc.vector.tensor_copy(sT_sb[:], sT_ps[:])
        nc.sync.dma_start(s_flat[0:1, hh * 1024 : (hh + 1) * 1024], sT_sb[:, :])
        for k in range(2):
            c = 2 * hh + k
            sl = slice(c * 512, (c + 1) * 512)
            S_ps = ps_tr.tile([128, 512], FP32, tag="tr2", bufs=2)
            nc.tensor.matmul(
                S_ps[:], lhsT=ones_row_b[:], rhs=s_flat[0:1, sl],
                start=True, stop=True,
            )
            nc.scalar.copy(S_row[:, sl], S_ps[:])
        nc.vector.tensor_scalar(neg_s[:, bs], s_mat[:, bs], -1.0, None, ALU.mult)
    for stat in (Pk, Pkh, Rk, Rkh):
        nc.gpsimd.memset(stat[:], 0.0)

    # p = exp(s), q = exp(0.2 s) on ACT
    p_v = vec_p.tile([128, NB], FP32, tag="p_v")
    nc.scalar.activation(p_v[:], s_mat[:], AFT.Exp)
    q_v = vec_p.tile([128, NB], FP32, tag="q_v")
    nc.scalar.activation(q_v[:], s_mat[:], AFT.Exp, scale=NEG_SLOPE)

    # ---------------- masks (emitted in consumption order) ----------------
    mask_tiles = [mask_p.tile([128, N], BF16, tag="mask", name=f"mask{a}")
                  for a in range(NB)]

    def mask_dve(a, lo, hi):
        nc.vector.tensor_scalar(
            mask_tiles[a][:, lo:hi], S_row[:, lo:hi], neg_s[:, a : a + 1], 0.5,
            ALU.is_gt, ALU.subtract,
        )

    # DVE: split blocks 0-3 (left halves start on S_row[:, :1024])
    for a in (0, 1, 2, 3):
        mask_dve(a, 0, 1024)

    # xT (bf16) via PE transposes, after the s products (the "tr" psum ring
    # recycles vbc4's bank, so these must come after all its readers);
    # copies on DVE between the mask waves
    for hh in range(2):
        tp = ps_tr.tile([128, 1024], BF16, tag="tr")
        for k in range(8):
            t = 8 * hh + k
            nc.tensor.matmul(
                tp[:, k * 128 : (k + 1) * 128], lhsT=x_bf[:, t, :], rhs=ident_b[:],
                is_transpose=True, start=(k % 2 == 0), stop=(k % 2 == 1),
            )
        nc.vector.tensor_copy(xT[:, 8 * hh : 8 * hh + 8, :], tp[:])

    for a in (0, 1, 2, 3):
        mask_dve(a, 1024, 2048)
    # ACT masks (emitted now so they start right after the exps)
    for a in (4, 9, 11, 14):
        nc.scalar.activation(
            mask_tiles[a][:], S_row[:], AFT.Sign, bias=s_mat[:, a : a + 1]
        )

    # Pk packing (DVE, after exps)
    nc.vector.tensor_copy(Pk[:, :, 0], p_v[:])
    p_hi32 = vec_p.tile([128, NB], FP32, tag="p_hi32")
    nc.vector.tensor_copy(p_hi32[:], Pk[:, :, 0])
    nc.vector.tensor_tensor(Pk[:, :, 1], p_v[:], p_hi32[:], ALU.subtract)
    nc.vector.tensor_scalar(Pk[:, :, 2], q_v[:], -1.0, None, ALU.mult)
    qn_hi32 = vec_p.tile([128, NB], FP32, tag="qn_hi32")
    nc.vector.tensor_copy(qn_hi32[:], Pk[:, :, 2])
    nc.vector.scalar_tensor_tensor(
        Pk[:, :, 3], qn_hi32[:], -1.0, q_v[:], ALU.mult, ALU.subtract
    )
    nc.vector.tensor_scalar(Pkh[:, :, 0:4], Pk[:, :, 0:4], 0.5, None, ALU.mult)

    # Ptot/Qtot -> half-total broadcast htot [128, 2] = [Ptot/2, Qtot/2]
    # (before the full-width masks so the PE-side matmuls don't stall the
    # PE queue behind the DVE mask stream)
    pq_s = vec_p.tile([128, 2], FP32, tag="pq_s")
    nc.vector.reduce_sum(pq_s[:, 0:1], p_v[:], axis=mybir.AxisListType.X)
    nc.vector.reduce_sum(pq_s[:, 1:2], q_v[:], axis=mybir.AxisListType.X)
    tot_ps = ps_sm.tile([1, 2], FP32, tag="sm")
    nc.tensor.matmul(tot_ps[:], lhsT=ones_f[:], rhs=pq_s[:], start=True, stop=True)
    tot_sb = vec_p.tile([1, 2], FP32, tag="tot_sb")
    nc.vector.tensor_copy(tot_sb[:], tot_ps[:])
    htot_ps = ps_sm.tile([128, 2], FP32, tag="sm")
    nc.tensor.matmul(
        htot_ps[:], lhsT=halves_row_f[:], rhs=tot_sb[:], start=True, stop=True
    )
    htot = vec_p.tile([128, 2], FP32, tag="htot")
    nc.vector.tensor_copy(htot[:], htot_ps[:])

    # remaining DVE masks, full width, in mv1 consumption order
    for a in (8, 5, 6, 7, 12, 13, 10, 15):
        mask_dve(a, 0, 2048)

    # ---------------- h = x @ W in [tok, f] orientation -------------------
    # lhsT = xT block (stationary), rhs = W_hi (moving); copies deferred so
    # they don't block the DVE mask stream
    h_sb = big_p.tile([128, NB, 128], BF16, tag="h_sb")
    h_ps_tiles = {}

    def h_mm(c):
        hp = ps_tr.tile([128, 4, 128], FP32, tag="tr2", bufs=2)
        h_ps_tiles[c] = hp
        for tt in range(4):
            t = 4 * c + tt
            nc.tensor.matmul(
                hp[:, tt, :], lhsT=xT[:, t, :], rhs=W_hi[:], start=True, stop=True
            )

    def h_copy(c):
        nc.vector.tensor_copy(h_sb[:, 4 * c : 4 * c + 4, :], h_ps_tiles[c][:])

    def stat_for(base, base_h, a):
        return (base_h if MASK_ENG[a] == "a" else base)[:, a, :]

    # ---------------- matvec 1 (col-tiled; last round c-outer) ------------
    d_ps = ps_big.tile([128, N], FP32, tag="bigps")

    def mv1_mm(a, c, stop):
        j = a % 4
        nc.tensor.matmul(
            d_ps[32 * j : 32 * j + 32, c * 512 : (c + 1) * 512],
            lhsT=stat_for(Pk, Pkh, a),
            rhs=mask_tiles[a][:, c * 512 : (c + 1) * 512],
            start=(a in MV1_WAVE1),
            stop=stop,
            tile_position=(0, 32 * j),
            skip_group_check=True,
        )

    # wave1 c-major over the split blocks: left chunks stream while the
    # right mask halves are still being generated; h matmuls fill the
    # PE while the full-width masks are produced
    for c in (0, 1):
        for a in MV1_WAVE1:
            mv1_mm(a, c, False)
    h_mm(0)
    for c in (2, 3):
        for a in MV1_WAVE1:
            mv1_mm(a, c, False)
    h_mm(1)
    for a in MV1_ORDER:
        for c in range(NC4):
            mv1_mm(a, c, False)

    # deferred h copies (DVE, after the mask stream)
    h_copy(0)
    h_copy(1)

    # per-chunk tail: copy psum chunk to SBUF, then one K=128 reduction
    # matmul per token block: csb_block^T @ W2 -> [128 tok, 2] partials
    Dt2_ps = ps_sm.tile([128, NB, 2], FP32, tag="sm")

    def mv_tail_chunk(src_ps, dst2_ps, c, copy1):
        csb = vec_p.tile([128, 512], FP32, tag=f"csb{c % 2}")
        copy1(csb[:], src_ps[:, c * 512 : (c + 1) * 512])
        for tt in range(4):
            t = c * 4 + tt
            nc.tensor.matmul(
                dst2_ps[:, t, :],
                lhsT=csb[:, tt * 128 : (tt + 1) * 128],
                rhs=W2[:],
                start=True,
                stop=True,
            )

    # tails one chunk behind the LAST rounds so the W2 matmuls never make
    # the PE wait on the csb copies
    def mv1_tail(c):
        mv_tail_chunk(
            d_ps, Dt2_ps, c,
            nc.vector.tensor_copy if c % 2 == 0 else nc.scalar.copy,
        )

    for c in range(NC4):
        for j in range(4):
            mv1_mm(MV1_LAST[j], c, True)
        if c >= 1:
            mv1_tail(c - 1)
    mv1_tail(3)

    # late h chunks fill the PE while the combine chain runs on DVE
    h_mm(2)
    h_mm(3)

    Dt2 = vec_p.tile([128, NB, 2], FP32, tag="Dt2")
    nc.vector.tensor_copy(Dt2[:], Dt2_ps[:])

    # ---------------- combine: A/B [128, NB], D, Rk ------------------------
    A_v = vec_p.tile([128, NB], FP32, tag="A_v")
    nc.vector.tensor_scalar(A_v[:], Dt2[:, :, 0], htot[:, 0:1], None, ALU.add)
    B_v = vec_p.tile([128, NB], FP32, tag="B_v")
    nc.vector.tensor_scalar(B_v[:], Dt2[:, :, 1], htot[:, 1:2], None, ALU.add)
    t1 = vec_p.tile([128, NB], FP32, tag="t1")
    nc.vector.tensor_tensor(t1[:], p_v[:], A_v[:], ALU.mult)
    t2 = vec_p.tile([128, NB], FP32, tag="t2")
    nc.vector.tensor_tensor(t2[:], q_v[:], B_v[:], ALU.mult)
    D_v = vec_p.tile([128, NB], FP32, tag="D_v")
    nc.vector.tensor_tensor(D_v[:], t1[:], t2[:], ALU.add)
    invD = vec_p.tile([128, NB], FP32, tag="invD")
    nc.vector.reciprocal(invD[:], D_v[:])
    r_v = vec_p.tile([128, NB], FP32, tag="r_v")
    nc.vector.tensor_tensor(r_v[:], p_v[:], invD[:], ALU.mult)
    u_v = vec_p.tile([128, NB], FP32, tag="u_v")
    nc.vector.tensor_tensor(u_v[:], q_v[:], invD[:], ALU.mult)
    nc.vector.tensor_copy(Rk[:, :, 0], r_v[:])
    r_hi32 = vec_p.tile([128, NB], FP32, tag="r_hi32")
    nc.vector.tensor_copy(r_hi32[:], Rk[:, :, 0])
    nc.vector.tensor_tensor(Rk[:, :, 1], r_v[:], r_hi32[:], ALU.subtract)
    nc.vector.tensor_scalar(Rk[:, :, 2], u_v[:], -1.0, None, ALU.mult)
    un_hi32 = vec_p.tile([128, NB], FP32, tag="un_hi32")
    nc.vector.tensor_copy(un_hi32[:], Rk[:, :, 2])
    nc.vector.scalar_tensor_tensor(
        Rk[:, :, 3], un_hi32[:], -1.0, u_v[:], ALU.mult, ALU.subtract
    )
    nc.vector.tensor_scalar(Rkh[:, :, 0:4], Rk[:, :, 0:4], 0.5, None, ALU.mult)

    # Rtot/Utot -> half-total broadcast htot2 [128, 2]
    ru_s = vec_p.tile([128, 2], FP32, tag="ru_s")
    nc.vector.reduce_sum(ru_s[:, 0:1], r_v[:], axis=mybir.AxisListType.X)
    nc.vector.reduce_sum(ru_s[:, 1:2], u_v[:], axis=mybir.AxisListType.X)
    tot2_ps = ps_sm.tile([1, 2], FP32, tag="sm")
    nc.tensor.matmul(tot2_ps[:], lhsT=ones_f[:], rhs=ru_s[:], start=True, stop=True)
    tot2_sb = vec_p.tile([1, 2], FP32, tag="tot2_sb")
    nc.vector.tensor_copy(tot2_sb[:], tot2_ps[:])
    htot2_ps = ps_sm.tile([128, 2], FP32, tag="sm")
    nc.tensor.matmul(
        htot2_ps[:], lhsT=halves_row_f[:], rhs=tot2_sb[:], start=True, stop=True
    )
    htot2 = vec_p.tile([128, 2], FP32, tag="htot2")
    nc.vector.tensor_copy(htot2[:], htot2_ps[:])

    # deferred h copies for chunks 2,3 (combine window, after the htot2
    # chain so they don't delay the mv2 start)
    h_copy(2)
    h_copy(3)

    # ---------------- matvec 2 (c-outer, col-tiled, pipelined tails) ------
    out_view = out_d.rearrange("(p t) f -> p t f", t=NB)
    g_ps = ps_big.tile([128, N], FP32, tag="bigps")
    Gt2_ps = ps_sm.tile([128, NB, 2], FP32, tag="sm")
    gsb = vec_p.tile([128, NB, 2], FP32, tag="gsb")
    gA = vec_p.tile([128, NB], FP32, tag="gA")
    gB = vec_p.tile([128, NB], FP32, tag="gB")
    gt1 = vec_p.tile([128, NB], FP32, tag="gt1")
    col = vec_p.tile([128, NB], FP32, tag="col")
    fin_tmp = vec_p.tile([128, 4, 128], FP32, tag="fin_tmp")

    def mv2_chunk(c):
        for rr in range(NC4):
            for j in range(4):
                a = 4 * rr + j
                nc.tensor.matmul(
                    g_ps[32 * j : 32 * j + 32, c * 512 : (c + 1) * 512],
                    lhsT=stat_for(Rk, Rkh, a),
                    rhs=mask_tiles[a][:, c * 512 : (c + 1) * 512],
                    start=(rr == 0),
                    stop=(rr == 3),
                    tile_position=(0, 32 * j),
                    skip_group_check=True,
                )

    def fin_block(o_sb, tt, t):
        """out_block = lrelu(h * col): ACT fused Prelu, or DVE pair."""
        eng = FIN_ENG[tt] if USE_PRELU else "v"
        if eng == "a":
            nc.scalar.activation(
                o_sb[:, tt, :], h_sb[:, t, :], AFT.Prelu,
                scale=col[:, t : t + 1], alpha=NEG_SLOPE,
            )
        else:
            tmp = fin_tmp[:, tt, :]
            nc.vector.tensor_scalar(
                tmp[:], h_sb[:, t, :], col[:, t : t + 1], None, ALU.mult
            )
            nc.vector.scalar_tensor_tensor(
                o_sb[:, tt, :], tmp[:], NEG_SLOPE, tmp[:], ALU.mult, ALU.max
            )

    def mv2_tail(c):
        mv_tail_chunk(
            g_ps, Gt2_ps, c,
            nc.vector.tensor_copy if c % 2 == 0 else nc.scalar.copy,
        )
        ts = slice(c * 4, (c + 1) * 4)
        nc.vector.tensor_copy(gsb[:, ts, :], Gt2_ps[:, ts, :])
        nc.vector.tensor_scalar(gA[:, ts], gsb[:, ts, 0], htot2[:, 0:1], None, ALU.add)
        nc.vector.tensor_scalar(gB[:, ts], gsb[:, ts, 1], htot2[:, 1:2], None, ALU.add)
        nc.vector.tensor_tensor(gt1[:, ts], p_v[:, ts], gA[:, ts], ALU.mult)
        nc.vector.tensor_tensor(col[:, ts], q_v[:, ts], gB[:, ts], ALU.mult)
        nc.vector.tensor_tensor(col[:, ts], col[:, ts], gt1[:, ts], ALU.add)
        o_sb = outsb_p.tile([128, 4, 128], FP32, tag="o_sb")
        for tt in range(4):
            fin_block(o_sb, tt, c * 4 + tt)
        (nc.sync if c % 2 == 0 else nc.scalar).dma_start(
            out_view[:, c * 4 : (c + 1) * 4, :], o_sb[:]
        )

    mv2_chunk(0)
    mv2_chunk(1)
    mv2_tail(0)
    mv2_chunk(2)
    mv2_tail(1)
    mv2_chunk(3)
    mv2_tail(2)
    mv2_tail(3)


def build_nc(num_devices: int = 8) -> "bass.Bass":
    nc = bacc.Bacc(
        "TRN2", target_bir_lowering=False, debug=False, num_devices=num_devices
    )
    x_d = nc.dram_tensor("x", [N, F], FP32, kind="ExternalInput")
    W_d = nc.dram_tensor("W", [F, F], FP32, kind="ExternalInput")
    wm_d = nc.dram_tensor("w_mlp", [F], FP32, kind="ExternalInput")
    bm_d = nc.dram_tensor("b_mlp", [1], FP32, kind="ExternalInput")
    out_d = nc.dram_tensor("out", [N, F], FP32, kind="ExternalOutput")
    with tile.TileContext(nc) as tc:
        with ExitStack() as ctx:
            gat_kernel(ctx, tc, out_d.ap(), x_d.ap(), W_d.ap(), wm_d.ap(), bm_d.ap())
    nc.compile()
    return nc


_NC_CACHE: dict = {}


def run(x, W, w_mlp, b_mlp, trace=False, **spmd_kwargs):
    x = np.asarray(x, dtype=np.float32)
    W = np.asarray(W, dtype=np.float32)
    w_mlp = np.asarray(w_mlp, dtype=np.float32)
    b_mlp = np.asarray(b_mlp, dtype=np.float32)

    if "nc" not in _NC_CACHE:
        _NC_CACHE["nc"] = build_nc(num_devices=B)
    nc = _NC_CACHE["nc"]

    in_maps = [
        {"x": np.ascontiguousarray(x[b, 0]), "W": W, "w_mlp": w_mlp, "b_mlp": b_mlp}
        for b in range(B)
    ]
    res = run_bass_kernel_spmd(
        nc, in_maps, core_ids=list(range(B)), trace=trace, **spmd_kwargs
    )
    out = np.stack([res.results[b]["out"] for b in range(B)])[:, None]
    return out.astype(np.float32), res


def kernel(x, W, w_mlp, b_mlp):
    out, _ = run(x, W, w_mlp, b_mlp)
    return out


# revision 5
# speedup vs baseline: 1.0755x; 1.0464x over previous
"""GAT layer kernel (v7.1) for Trainium2 (Bass/Tile), data-parallel over batch on 8 cores.

v7.1 vs v6: s computed on DVE (fp32 mult+reduce vs v-broadcast); x
transposed once in bf16; h computed directly in [tok, f] orientation
(xT-block stationary x W moving) killing the fp32 lrl transposes; final
out = Prelu(h*col) fused on ACT (scale=col); token layout (p t) so x/out
DMAs move 2KB+ contiguous per partition; engine queues ordered so masks
start as early as possible (DVE 12 + ACT 4); h matmul chunks 2,3 fill
the PE during the combine phase.

Per-core computation (batch b, N=2048, F=128):
    s = x @ (W @ w_mlp) + b;  p = exp(s), q = exp(0.2 s)
    mt[i,j] = [s_i + s_j > 0] - 1/2   (symmetric, values +-1/2)
    D_i = p_i ((mt p)_i + Ptot/2) + q_i ((mt (-q))_i + Qtot/2)
    col  = p ((mt r) + Rtot/2) + q ((mt (-u)) + Utot/2),  r = p/D, u = q/D
    out  = lrelu(h * col),  h = x @ W
"""

import sys

if "/opt/trn_rl_repo" not in sys.path:
    sys.path.insert(0, "/opt/trn_rl_repo")

from contextlib import ExitStack

import numpy as np

import concourse.bass as bass
import concourse.mybir as mybir
import concourse.tile as tile
from concourse import bacc
from concourse import masks
from concourse.bass_utils import run_bass_kernel_spmd

B, N, F = 8, 2048, 128
NB = N // 128  # 16 token blocks
NC4 = 4  # 512-wide chunks
NEG_SLOPE = 0.2
FP32 = mybir.dt.float32
BF16 = mybir.dt.bfloat16
ALU = mybir.AluOpType
AFT = mybir.ActivationFunctionType

# mask block -> engine: "v" DVE (is_gt - 0.5), "a" ACT (Sign, +-1, halved
# stationary). Emission order matches mv1 consumption readiness.
MASK_ENG = {a: "v" for a in range(NB)}
for a in (4, 9, 11, 14, 15):
    MASK_ENG[a] = "a"
# wave1 (blocks 0-3, split masks) runs c-major and is the start of every
# accumulation region; LAST = the ACT-generated masks (ready earliest of
# the late blocks) consumed c-outer with stop
MV1_WAVE1 = [0, 1, 2, 3]
MV1_ORDER = [8, 5, 6, 7, 12, 13, 10, 15]
MV1_LAST = [4, 9, 14, 11]
# ACT Prelu not implemented in CoreSim; sim_check flips this off.
USE_PRELU = True
# final out blocks engine split (t % 4): "a" ACT Prelu, "v" DVE pair
FIN_ENG = {0: "a", 1: "v", 2: "a", 3: "v"}


def gat_kernel(ctx: ExitStack, tc: "tile.TileContext", out_d, x_d, W_d, wm_d, bm_d):
    nc = tc.nc

    const_p = ctx.enter_context(tc.tile_pool(name="const", bufs=1))
    big_p = ctx.enter_context(tc.tile_pool(name="big", bufs=1))
    mask_p = ctx.enter_context(tc.tile_pool(name="mask", bufs=NB))
    vec_p = ctx.enter_context(tc.tile_pool(name="vec", bufs=1))
    outsb_p = ctx.enter_context(tc.tile_pool(name="outsb", bufs=4))
    # PSUM: bigps 4 banks + tr 1 + tr2 2x1 + sm 1 = 8
    ps_big = ctx.enter_context(tc.tile_pool(name="ps_big", bufs=1, space="PSUM"))
    ps_tr = ctx.enter_context(tc.tile_pool(name="ps_tr", bufs=1, space="PSUM"))
    ps_sm = ctx.enter_context(tc.tile_pool(name="ps_sm", bufs=1, space="PSUM"))

    # iota first on gpsimd: nothing depends upstream and it unblocks the
    # W2 setup on DVE before the s chain claims the queue
    ioti = const_p.tile([128, 1], mybir.dt.int32, tag="ioti")
    nc.gpsimd.iota(ioti[:], [[0, 1]], base=0, channel_multiplier=1)

    # ---------------- input DMAs first (x is the critical path) ----------
    # token layout: tok = p*16 + t  ->  per-partition 2KB-contiguous DMAs
    x_view = x_d.rearrange("(p t) f -> p t f", t=NB)
    x_sb = big_p.tile([128, NB, 128], FP32, tag="x_sb")
    W_sb = const_p.tile([128, 128], FP32, tag="W_sb")
    nc.sync.dma_start(W_sb[:], W_d[:, :])
    wm_sb = const_p.tile([128, 1], FP32, tag="wm_sb")
    nc.scalar.dma_start(wm_sb[:], wm_d.rearrange("(p o) -> p o", o=1))
    b_sb = const_p.tile([1, 1], FP32, tag="b_sb")
    nc.scalar.dma_start(b_sb[:], bm_d.rearrange("(p o) -> p o", o=1))
    nc.sync.dma_start(x_sb[:, 0:4, :], x_view[:, 0:4, :])
    nc.scalar.dma_start(x_sb[:, 4:8, :], x_view[:, 4:8, :])
    nc.sync.dma_start(x_sb[:, 8:12, :], x_view[:, 8:12, :])
    # x3 on scalar, outputs on sync: gpsimd issues no DMAs at all, so its
    # (slow, ~2.5us) software-DGE drain never sits at the end of the kernel
    nc.scalar.dma_start(x_sb[:, 12:16, :], x_view[:, 12:16, :])
    # identities next on the gpsimd queue: ident_f gates the early W
    # transpose, ident_b the bf16 transposes
    ident_f = const_p.tile([128, 128], FP32, tag="ident_f")
    ident_b = const_p.tile([128, 128], BF16, tag="ident_b")
    masks.make_identity(nc, ident_f[:])
    masks.make_identity(nc, ident_b[:])

    # ---------------- constants ----------------
    ones_f = const_p.tile([128, 1], FP32, tag="ones_f")
    nc.gpsimd.memset(ones_f[:], 1.0)
    ones_row_f = const_p.tile([1, 128], FP32, tag="ones_row_f")
    nc.gpsimd.memset(ones_row_f[:], 1.0)
    halves_row_f = const_p.tile([1, 128], FP32, tag="halves_row_f")
    nc.gpsimd.memset(halves_row_f[:], 0.5)
    ones_row_b = const_p.tile([1, 128], BF16, tag="ones_row_b")
    nc.gpsimd.memset(ones_row_b[:], 1.0)
    # zero-padded 32-col stationaries (cols 4..31 stay 0 so every col-group
    # writes its full 32-partition PSUM range); memsets run on gpsimd after
    # the x casts so they don't delay the transposes
    Pk = vec_p.tile([128, NB, 32], BF16, tag="Pk")
    Pkh = vec_p.tile([128, NB, 32], BF16, tag="Pkh")
    Rk = vec_p.tile([128, NB, 32], BF16, tag="Rk")
    Rkh = vec_p.tile([128, NB, 32], BF16, tag="Rkh")

    # W2 [128, 2]: col0 selects rows {32j+0,32j+1}, col1 rows {32j+2,32j+3}
    W2 = const_p.tile([128, 2], FP32, tag="W2")
    m32i = const_p.tile([128, 1], mybir.dt.int32, tag="m32i")
    nc.vector.tensor_scalar(m32i[:], ioti[:], 31, None, ALU.bitwise_and)
    m32 = const_p.tile([128, 1], FP32, tag="m32")
    nc.vector.tensor_copy(m32[:], m32i[:])
    nc.vector.tensor_scalar(W2[:, 0:1], m32[:], 2.0, None, ALU.is_lt)
    lt4 = const_p.tile([128, 1], FP32, tag="lt4")
    nc.vector.tensor_scalar(lt4[:], m32[:], 4.0, None, ALU.is_lt)
    nc.vector.tensor_tensor(W2[:, 1:2], lt4[:], W2[:, 0:1], ALU.subtract)

    # Preload the ACT table set early (exp_and_others also holds sign, copy,
    # parametric_relu)
    warm = const_p.tile([128, 2], FP32, tag="warm")
    nc.scalar.activation(warm[:, 0:1], ones_f[:], AFT.Exp)
    nc.scalar.activation(warm[:, 1:2], ones_f[:], AFT.Sign)

    # ---------------- v row = (W @ w_mlp)^T, broadcast to [128, 4, 128] --
    # vrow[f] = sum_g wm[g] W[f, g] = wm^T @ W^T
    WT_ps = ps_sm.tile([128, 128], FP32, tag="sm")
    nc.tensor.transpose(WT_ps[:], W_sb[:], ident_f[:])
    WT_sb = vec_p.tile([128, 128], FP32, tag="WT_sb")
    nc.vector.tensor_copy(WT_sb[:], WT_ps[:])
    vrow_ps = ps_sm.tile([1, 128], FP32, tag="sm")
    nc.tensor.matmul(vrow_ps[:], lhsT=wm_sb[:], rhs=WT_sb[:], start=True, stop=True)
    vrow_sb = vec_p.tile([1, 128], FP32, tag="vrow_sb")
    nc.vector.tensor_copy(vrow_sb[:], vrow_ps[:])
    # v broadcast down partitions (single width; the s products read it
    # from PSUM through a stride-0 x4 view)
    vbc_ps = ps_tr.tile([128, 128], FP32, tag="tr")
    nc.tensor.matmul(
        vbc_ps[:], lhsT=ones_row_f[:], rhs=vrow_sb[:], start=True, stop=True
    )
    vbc4_view = vbc_ps.rearrange("p (o b) -> p o b", o=1).broadcast_to(
        (128, 4, 128)
    )

    # b broadcast to [128,1] via K=1 PE matmul
    b_ps = ps_sm.tile([128, 1], FP32, tag="sm")
    nc.tensor.matmul(b_ps[:], lhsT=ones_row_f[:], rhs=b_sb[:], start=True, stop=True)
    b_bc = const_p.tile([128, 1], FP32, tag="b_bc")
    nc.vector.tensor_copy(b_bc[:], b_ps[:])

    # W in bf16 (single pass; rel-err budget 2e-2 allows it)
    W_hi = const_p.tile([128, 128], BF16, tag="W_hi")
    nc.scalar.copy(W_hi[:], W_sb[:])

    # ---------------- per-half: cast x, s on DVE, s-chain, xT -------------
    x_bf = big_p.tile([128, NB, 128], BF16, tag="x_bf")
    xT = big_p.tile([128, NB, 128], BF16, tag="xT")
    s_mat = vec_p.tile([128, NB], FP32, tag="s_mat")
    s_prod = vec_p.tile([128, 4, 128], FP32, tag="s_prod")
    s_hi = vec_p.tile([128, NB], BF16, tag="s_hi")
    neg_s = vec_p.tile([128, NB], FP32, tag="neg_s")
    s_flat = vec_p.tile([1, N], BF16, tag="s_flat")
    S_row = big_p.tile([128, N], BF16, tag="S_row")

    def eng_copy(e, dst, src):
        (e.copy if e is nc.scalar else e.tensor_copy)(dst, src)

    cast_eng = [nc.vector, nc.scalar, nc.vector, nc.scalar]

    for hh in range(2):
        for c in (2 * hh, 2 * hh + 1):
            # s products first: the casts must not head-of-line-block them
            cs = slice(4 * c, 4 * c + 4)
            nc.vector.tensor_tensor(s_prod[:], x_sb[:, cs, :], vbc4_view, ALU.mult)
            nc.vector.reduce_sum(s_mat[:, cs], s_prod[:], axis=mybir.AxisListType.X)
            for h2 in range(2):
                sl = slice(4 * c + 2 * h2, 4 * c + 2 * h2 + 2)
                eng_copy(cast_eng[(2 * c + h2) % 4], x_bf[:, sl, :], x_sb[:, sl, :])
        # s chain: s_flat row + S_row broadcast for this half
        bs = slice(hh * 8, (hh + 1) * 8)
        nc.vector.tensor_scalar(
            s_mat[:, bs], s_mat[:, bs], b_bc[:, 0:1], None, ALU.add
        )
        nc.vector.tensor_copy(s_hi[:, bs], s_mat[:, bs])
        sT_ps = ps_sm.tile([8, 128], BF16, tag="sm")
        nc.tensor.transpose(sT_ps[:], s_hi[:, bs], ident_b[:])
        sT_sb = vec_p.tile([8, 128], BF16, tag=f"sT_sb{hh}")
        nc.vector.tensor_copy(sT_sb[:], sT_ps[:])
        nc.sync.dma_start(s_flat[0:1, hh * 1024 : (hh + 1) * 1024], sT_sb[:, :])
        # S_row via one gpsimd partition-broadcast: drops the PE-bcast +
        # ACT-copy hops and frees ACT to take a fifth mask block
        nc.gpsimd.partition_broadcast(
            S_row[:, hh * 1024 : (hh + 1) * 1024],
            s_flat[0:1, hh * 1024 : (hh + 1) * 1024],
        )
        nc.vector.tensor_scalar(neg_s[:, bs], s_mat[:, bs], -1.0, None, ALU.mult)
    for stat in (Pk, Pkh, Rk, Rkh):
        nc.gpsimd.memset(stat[:], 0.0)

    # p = exp(s), q = exp(0.2 s) on ACT
    p_v = vec_p.tile([128, NB], FP32, tag="p_v")
    nc.scalar.activation(p_v[:], s_mat[:], AFT.Exp)
    q_v = vec_p.tile([128, NB], FP32, tag="q_v")
    nc.scalar.activation(q_v[:], s_mat[:], AFT.Exp, scale=NEG_SLOPE)

    # ---------------- masks (emitted in consumption order) ----------------
    mask_tiles = [mask_p.tile([128, N], BF16, tag="mask", name=f"mask{a}")
                  for a in range(NB)]

    def mask_dve(a, lo, hi):
        nc.vector.tensor_scalar(
            mask_tiles[a][:, lo:hi], S_row[:, lo:hi], neg_s[:, a : a + 1], 0.5,
            ALU.is_gt, ALU.subtract,
        )

    # DVE: split blocks 0-3 (left halves start on S_row[:, :1024])
    for a in (0, 1, 2, 3):
        mask_dve(a, 0, 1024)

    # xT (bf16) via PE transposes, after the s products (the "tr" psum ring
    # recycles vbc4's bank, so these must come after all its readers);
    # copies on DVE between the mask waves
    for hh in range(2):
        tp = ps_tr.tile([128, 1024], BF16, tag="tr")
        for k in range(8):
            t = 8 * hh + k
            nc.tensor.matmul(
                tp[:, k * 128 : (k + 1) * 128], lhsT=x_bf[:, t, :], rhs=ident_b[:],
                is_transpose=True, start=(k % 2 == 0), stop=(k % 2 == 1),
            )
        nc.vector.tensor_copy(xT[:, 8 * hh : 8 * hh + 8, :], tp[:])

    for a in (0, 1, 2, 3):
        mask_dve(a, 1024, 2048)
    # ACT masks (emitted now so they start right after the exps)
    for a in (4, 9, 15, 14, 11):
        nc.scalar.activation(
            mask_tiles[a][:], S_row[:], AFT.Sign, bias=s_mat[:, a : a + 1]
        )

    # Pk packing (DVE, after exps)
    nc.vector.tensor_copy(Pk[:, :, 0], p_v[:])
    p_hi32 = vec_p.tile([128, NB], FP32, tag="p_hi32")
    nc.vector.tensor_copy(p_hi32[:], Pk[:, :, 0])
    nc.vector.tensor_tensor(Pk[:, :, 1], p_v[:], p_hi32[:], ALU.subtract)
    nc.vector.tensor_scalar(Pk[:, :, 2], q_v[:], -1.0, None, ALU.mult)
    qn_hi32 = vec_p.tile([128, NB], FP32, tag="qn_hi32")
    nc.vector.tensor_copy(qn_hi32[:], Pk[:, :, 2])
    nc.vector.scalar_tensor_tensor(
        Pk[:, :, 3], qn_hi32[:], -1.0, q_v[:], ALU.mult, ALU.subtract
    )
    nc.vector.tensor_scalar(Pkh[:, :, 0:4], Pk[:, :, 0:4], 0.5, None, ALU.mult)

    # Ptot/Qtot -> half-total broadcast htot [128, 2] = [Ptot/2, Qtot/2]
    # (before the full-width masks so the PE-side matmuls don't stall the
    # PE queue behind the DVE mask stream)
    pq_s = vec_p.tile([128, 2], FP32, tag="pq_s")
    nc.vector.reduce_sum(pq_s[:, 0:1], p_v[:], axis=mybir.AxisListType.X)
    nc.vector.reduce_sum(pq_s[:, 1:2], q_v[:], axis=mybir.AxisListType.X)
    tot_ps = ps_sm.tile([1, 2], FP32, tag="sm")
    nc.tensor.matmul(tot_ps[:], lhsT=ones_f[:], rhs=pq_s[:], start=True, stop=True)
    tot_sb = vec_p.tile([1, 2], FP32, tag="tot_sb")
    nc.vector.tensor_copy(tot_sb[:], tot_ps[:])
    htot_ps = ps_sm.tile([128, 2], FP32, tag="sm")
    nc.tensor.matmul(
        htot_ps[:], lhsT=halves_row_f[:], rhs=tot_sb[:], start=True, stop=True
    )
    htot = vec_p.tile([128, 2], FP32, tag="htot")
    nc.vector.tensor_copy(htot[:], htot_ps[:])

    # remaining DVE masks, full width, in mv1 consumption order
    for a in (8, 5, 6, 7, 12, 13, 10):
        mask_dve(a, 0, 2048)

    # ---------------- h = x @ W in [tok, f] orientation -------------------
    # lhsT = xT block (stationary), rhs = W_hi (moving); copies deferred so
    # they don't block the DVE mask stream
    h_sb = big_p.tile([128, NB, 128], BF16, tag="h_sb")
    h_ps_tiles = {}

    def h_mm(c):
        hp = ps_tr.tile([128, 4, 128], FP32, tag="tr2", bufs=2)
        h_ps_tiles[c] = hp
        for tt in range(4):
            t = 4 * c + tt
            nc.tensor.matmul(
                hp[:, tt, :], lhsT=xT[:, t, :], rhs=W_hi[:], start=True, stop=True
            )

    def h_copy(c):
        nc.vector.tensor_copy(h_sb[:, 4 * c : 4 * c + 4, :], h_ps_tiles[c][:])

    def stat_for(base, base_h, a):
        return (base_h if MASK_ENG[a] == "a" else base)[:, a, :]

    # ---------------- matvec 1 (col-tiled; last round c-outer) ------------
    d_ps = ps_big.tile([128, N], FP32, tag="bigps")

    def mv1_mm(a, c, stop):
        j = a % 4
        nc.tensor.matmul(
            d_ps[32 * j : 32 * j + 32, c * 512 : (c + 1) * 512],
            lhsT=stat_for(Pk, Pkh, a),
            rhs=mask_tiles[a][:, c * 512 : (c + 1) * 512],
            start=(a in MV1_WAVE1),
            stop=stop,
            tile_position=(0, 32 * j),
            skip_group_check=True,
        )

    # wave1 c-major over the split blocks: left chunks stream while the
    # right mask halves are still being generated; h matmuls fill the
    # PE while the full-width masks are produced
    for c in (0, 1):
        for a in MV1_WAVE1:
            mv1_mm(a, c, False)
    h_mm(0)
    for c in (2, 3):
        for a in MV1_WAVE1:
            mv1_mm(a, c, False)
    h_mm(1)
    for a in MV1_ORDER:
        for c in range(NC4):
            mv1_mm(a, c, False)

    # deferred h copies (DVE, after the mask stream)
    h_copy(0)
    h_copy(1)

    # per-chunk tail: copy psum chunk to SBUF, then one K=128 reduction
    # matmul per token block: csb_block^T @ W2 -> [128 tok, 2] partials
    Dt2_ps = ps_sm.tile([128, NB, 2], FP32, tag="sm")

    def mv_tail_chunk(src_ps, dst2_ps, c, copy1):
        csb = vec_p.tile([128, 512], FP32, tag=f"csb{c % 2}")
        copy1(csb[:], src_ps[:, c * 512 : (c + 1) * 512])
        for tt in range(4):
            t = c * 4 + tt
            nc.tensor.matmul(
                dst2_ps[:, t, :],
                lhsT=csb[:, tt * 128 : (tt + 1) * 128],
                rhs=W2[:],
                start=True,
                stop=True,
            )

    # tails one chunk behind the LAST rounds so the W2 matmuls never make
    # the PE wait on the csb copies
    def mv1_tail(c):
        mv_tail_chunk(
            d_ps, Dt2_ps, c,
            nc.vector.tensor_copy if c % 2 == 0 else nc.scalar.copy,
        )

    for c in range(NC4):
        for j in range(4):
            mv1_mm(MV1_LAST[j], c, True)
        if c >= 1:
            mv1_tail(c - 1)
    mv1_tail(3)

    # late h chunks fill the PE while the combine chain runs on DVE
    h_mm(2)
    h_mm(3)

    Dt2 = vec_p.tile([128, NB, 2], FP32, tag="Dt2")
    nc.vector.tensor_copy(Dt2[:], Dt2_ps[:])

    # ---------------- combine: A/B [128, NB], D, Rk ------------------------
    A_v = vec_p.tile([128, NB], FP32, tag="A_v")
    nc.vector.tensor_scalar(A_v[:], Dt2[:, :, 0], htot[:, 0:1], None, ALU.add)
    B_v = vec_p.tile([128, NB], FP32, tag="B_v")
    nc.vector.tensor_scalar(B_v[:], Dt2[:, :, 1], htot[:, 1:2], None, ALU.add)
    t1 = vec_p.tile([128, NB], FP32, tag="t1")
    nc.vector.tensor_tensor(t1[:], p_v[:], A_v[:], ALU.mult)
    t2 = vec_p.tile([128, NB], FP32, tag="t2")
    nc.vector.tensor_tensor(t2[:], q_v[:], B_v[:], ALU.mult)
    D_v = vec_p.tile([128, NB], FP32, tag="D_v")
    nc.vector.tensor_tensor(D_v[:], t1[:], t2[:], ALU.add)
    invD = vec_p.tile([128, NB], FP32, tag="invD")
    nc.vector.reciprocal(invD[:], D_v[:])
    r_v = vec_p.tile([128, NB], FP32, tag="r_v")
    nc.vector.tensor_tensor(r_v[:], p_v[:], invD[:], ALU.mult)
    u_v = vec_p.tile([128, NB], FP32, tag="u_v")
    nc.vector.tensor_tensor(u_v[:], q_v[:], invD[:], ALU.mult)
    nc.vector.tensor_copy(Rk[:, :, 0], r_v[:])
    r_hi32 = vec_p.tile([128, NB], FP32, tag="r_hi32")
    nc.vector.tensor_copy(r_hi32[:], Rk[:, :, 0])
    nc.vector.tensor_tensor(Rk[:, :, 1], r_v[:], r_hi32[:], ALU.subtract)
    nc.vector.tensor_scalar(Rk[:, :, 2], u_v[:], -1.0, None, ALU.mult)
    un_hi32 = vec_p.tile([128, NB], FP32, tag="un_hi32")
    nc.vector.tensor_copy(un_hi32[:], Rk[:, :, 2])
    nc.vector.scalar_tensor_tensor(
        Rk[:, :, 3], un_hi32[:], -1.0, u_v[:], ALU.mult, ALU.subtract
    )
    nc.vector.tensor_scalar(Rkh[:, :, 0:4], Rk[:, :, 0:4], 0.5, None, ALU.mult)

    # Rtot/Utot -> half-total broadcast htot2 [128, 2]
    ru_s = vec_p.tile([128, 2], FP32, tag="ru_s")
    nc.vector.reduce_sum(ru_s[:, 0:1], r_v[:], axis=mybir.AxisListType.X)
    nc.vector.reduce_sum(ru_s[:, 1:2], u_v[:], axis=mybir.AxisListType.X)
    tot2_ps = ps_sm.tile([1, 2], FP32, tag="sm")
    nc.tensor.matmul(tot2_ps[:], lhsT=ones_f[:], rhs=ru_s[:], start=True, stop=True)
    tot2_sb = vec_p.tile([1, 2], FP32, tag="tot2_sb")
    nc.vector.tensor_copy(tot2_sb[:], tot2_ps[:])
    htot2_ps = ps_sm.tile([128, 2], FP32, tag="sm")
    nc.tensor.matmul(
        htot2_ps[:], lhsT=halves_row_f[:], rhs=tot2_sb[:], start=True, stop=True
    )
    htot2 = vec_p.tile([128, 2], FP32, tag="htot2")
    nc.vector.tensor_copy(htot2[:], htot2_ps[:])

    # deferred h copies for chunks 2,3 (combine window, after the htot2
    # chain so they don't delay the mv2 start)
    h_copy(2)
    h_copy(3)

    # ---------------- matvec 2 (c-outer, col-tiled, pipelined tails) ------
    out_view = out_d.rearrange("(p t) f -> p t f", t=NB)
    g_ps = ps_big.tile([128, N], FP32, tag="bigps")
    Gt2_ps = ps_sm.tile([128, NB, 2], FP32, tag="sm")
    gsb = vec_p.tile([128, NB, 2], FP32, tag="gsb")
    gA = vec_p.tile([128, NB], FP32, tag="gA")
    gB = vec_p.tile([128, NB], FP32, tag="gB")
    gt1 = vec_p.tile([128, NB], FP32, tag="gt1")
    col = vec_p.tile([128, NB], FP32, tag="col")
    fin_tmp = vec_p.tile([128, 4, 128], FP32, tag="fin_tmp")

    def mv2_chunk(c):
        for rr in range(NC4):
            for j in range(4):
                a = 4 * rr + j
                nc.tensor.matmul(
                    g_ps[32 * j : 32 * j + 32, c * 512 : (c + 1) * 512],
                    lhsT=stat_for(Rk, Rkh, a),
                    rhs=mask_tiles[a][:, c * 512 : (c + 1) * 512],
                    start=(rr == 0),
                    stop=(rr == 3),
                    tile_position=(0, 32 * j),
                    skip_group_check=True,
                )

    def fin_block(o_sb, tt, t):
        """out_block = lrelu(h * col): ACT fused Prelu, or DVE pair."""
        eng = FIN_ENG[tt] if USE_PRELU else "v"
        if eng == "a":
            nc.scalar.activation(
                o_sb[:, tt, :], h_sb[:, t, :], AFT.Prelu,
                scale=col[:, t : t + 1], alpha=NEG_SLOPE,
            )
        else:
            tmp = fin_tmp[:, tt, :]
            nc.vector.tensor_scalar(
                tmp[:], h_sb[:, t, :], col[:, t : t + 1], None, ALU.mult
            )
            nc.vector.scalar_tensor_tensor(
                o_sb[:, tt, :], tmp[:], NEG_SLOPE, tmp[:], ALU.mult, ALU.max
            )

    def mv2_tail(c):
        mv_tail_chunk(
            g_ps, Gt2_ps, c,
            nc.vector.tensor_copy if c % 2 == 0 else nc.scalar.copy,
        )
        ts = slice(c * 4, (c + 1) * 4)
        nc.vector.tensor_copy(gsb[:, ts, :], Gt2_ps[:, ts, :])
        nc.vector.tensor_scalar(gA[:, ts], gsb[:, ts, 0], htot2[:, 0:1], None, ALU.add)
        nc.vector.tensor_scalar(gB[:, ts], gsb[:, ts, 1], htot2[:, 1:2], None, ALU.add)
        nc.vector.tensor_tensor(gt1[:, ts], p_v[:, ts], gA[:, ts], ALU.mult)
        nc.vector.tensor_tensor(col[:, ts], q_v[:, ts], gB[:, ts], ALU.mult)
        nc.vector.tensor_tensor(col[:, ts], col[:, ts], gt1[:, ts], ALU.add)
        o_sb = outsb_p.tile([128, 4, 128], FP32, tag="o_sb")
        for tt in range(4):
            fin_block(o_sb, tt, c * 4 + tt)
        (nc.sync if c % 2 == 0 else nc.scalar).dma_start(
            out_view[:, c * 4 : (c + 1) * 4, :], o_sb[:]
        )

    mv2_chunk(0)
    mv2_chunk(1)
    mv2_tail(0)
    mv2_chunk(2)
    mv2_tail(1)
    mv2_chunk(3)
    mv2_tail(2)
    mv2_tail(3)


def build_nc(num_devices: int = 8) -> "bass.Bass":
    nc = bacc.Bacc(
        "TRN2", target_bir_lowering=False, debug=False, num_devices=num_devices
    )
    x_d = nc.dram_tensor("x", [N, F], FP32, kind="ExternalInput")
    W_d = nc.dram_tensor("W", [F, F], FP32, kind="ExternalInput")
    wm_d = nc.dram_tensor("w_mlp", [F], FP32, kind="ExternalInput")
    bm_d = nc.dram_tensor("b_mlp", [1], FP32, kind="ExternalInput")
    out_d = nc.dram_tensor("out", [N, F], FP32, kind="ExternalOutput")
    with tile.TileContext(nc) as tc:
        with ExitStack() as ctx:
            gat_kernel(ctx, tc, out_d.ap(), x_d.ap(), W_d.ap(), wm_d.ap(), bm_d.ap())
    nc.compile()
    return nc


_NC_CACHE: dict = {}


def run(x, W, w_mlp, b_mlp, trace=False, **spmd_kwargs):
    x = np.asarray(x, dtype=np.float32)
    W = np.asarray(W, dtype=np.float32)
    w_mlp = np.asarray(w_mlp, dtype=np.float32)
    b_mlp = np.asarray(b_mlp, dtype=np.float32)

    if "nc" not in _NC_CACHE:
        _NC_CACHE["nc"] = build_nc(num_devices=B)
    nc = _NC_CACHE["nc"]

    in_maps = [
        {"x": np.ascontiguousarray(x[b, 0]), "W": W, "w_mlp": w_mlp, "b_mlp": b_mlp}
        for b in range(B)
    ]
    res = run_bass_kernel_spmd(
        nc, in_maps, core_ids=list(range(B)), trace=trace, **spmd_kwargs
    )
    out = np.stack([res.results[b]["out"] for b in range(B)])[:, None]
    return out.astype(np.float32), res


def kernel(x, W, w_mlp, b_mlp):
    out, _ = run(x, W, w_mlp, b_mlp)
    return out
